# revision 3
# baseline (speedup 1.0000x reference)
"""AdaptiveGaussianConvLayer Trainium2 kernel (8 NeuronCores, SPMD, no collectives).

Math: out[b, j, d] = sum_i V[b, i, d] * W[b, i, j],
      W[b, i, j] = exp(-0.5 * ((j - i - mu[b,i]) / sigma[b,i])^2)
with B=4, N=4096, D=512; sigma in (0.5, 2.5), mu ~ 3*N(0,1).

Key structure: W underflows to exactly 0.0 in fp32 once |j - i - mu| / sigma
>= ~13.2 (exp(-87.4) -> 0), i.e. for |j - i| >= ~48.  We keep a generous
margin: each 128-wide j-tile accumulates over the 3 aligned 128-row i-slabs
centered on it (band of 384), which covers |j - i| <= 128; everything outside
is exactly zero in fp32, so the result matches the dense reference to fp32
rounding.

Sharding: 8 cores = (batch b in 0..3) x (j-half h in 0..1).  Core c computes
out[b, h*2048:(h+1)*2048, :].  The host pads V/sigma/mu with 128 zero rows on
each side of the core's i-window so every core runs the identical program
(pad rows have V=0 so they contribute nothing regardless of W).

On-device per core:
  - DMA in: Vp [2304, 512] f32 (18 slabs of [128, 512]), sb [128, 36]
    (per-slab per-partition scale r=1/sigma and bias b0=(-128-p-mu)*r),
    iota [128, 384] (row 0..383 in every partition).
  - Per slab s: ACT Square(iota * r_s + b0_s) -> z2; ACT Exp(-0.5 * z2) -> W_s
    [128 i-part, 384 j-free].  (z = u - 128 - p - mu)/sigma where u is the
    iota value; the j-window of slab s is j_local in [128(s-2), 128(s+2)).)
  - Per j-tile t (16): 3 matmuls accumulate PSUM[128 j, 512 d] over slabs
    ls = t, t+1, t+2 with lhsT = W_ls[:, (2-k)*128:(3-k)*128], rhs = V_ls.
  - DVE copy PSUM -> SBUF, DMA out tile -> out[t*128:(t+1)*128, :].
"""

import os
import numpy as np

import concourse.bass as bass
import concourse.bacc as bacc
import concourse.mybir as mybir
import concourse.tile as tile
from concourse.bass_utils import run_bass_kernel_spmd

AF = mybir.ActivationFunctionType

B, N, D = 4, 4096, 512
NCORES = 8
HALF = N // 2            # 2048 j per core
NSLAB = HALF // 128 + 2  # 18 i-slabs of 128 rows (1 pad slab each side)
VROWS = NSLAB * 128      # 2304
JT = HALF // 128         # 16 j-tiles per core
WWIN = 384               # j-window width per slab

# matmul compute dtype: "f32" (exact, 4 cyc/row), "f32r" (fast fp32 mode),
# "bf16" (fast, ~4e-3 rel err)
DT_MM = os.environ.get("AGC_DT_MM", "f32r")

_cached = {}


def build_nc(dt_mm: str = DT_MM):
    f32 = mybir.dt.float32
    nc = bacc.Bacc("TRN2", target_bir_lowering=False, debug=False)

    vp_d = nc.dram_tensor("Vp", [VROWS, D], f32, kind="ExternalInput").ap()
    sb_d = nc.dram_tensor("sb", [128, 2 * NSLAB], f32, kind="ExternalInput").ap()
    iota_d = nc.dram_tensor("iota", [128, WWIN], f32, kind="ExternalInput").ap()
    out_d = nc.dram_tensor("out", [HALF, D], f32, kind="ExternalOutput").ap()

    if dt_mm == "bf16":
        mm_dt = mybir.dt.bfloat16
    elif dt_mm == "f32r":
        mm_dt = mybir.dt.float32r
    else:
        mm_dt = f32

    with tile.TileContext(nc) as tc:
        with (
            tc.tile_pool(name="const", bufs=1) as constp,
            tc.tile_pool(name="vslab", bufs=NSLAB) as vpool,
            tc.tile_pool(name="wtile", bufs=NSLAB) as wpool,
            tc.tile_pool(name="z2", bufs=3) as zpool,
            tc.tile_pool(name="ps", bufs=4, space=bass.MemorySpace.PSUM) as pspool,
            tc.tile_pool(name="obuf", bufs=4) as opool,
        ):
            iota_t = constp.tile([128, WWIN], f32)
            sb_t = constp.tile([128, 2 * NSLAB], f32)
            nc.sync.dma_start(iota_t[:], iota_d[:])
            nc.sync.dma_start(sb_t[:], sb_d[:])

            vts = []
            vp3 = vp_d.rearrange("(s p) d -> s p d", p=128)
            for s in range(NSLAB):
                vt = vpool.tile([128, D], f32)
                nc.sync.dma_start(vt[:], vp3[s])
                if dt_mm in ("bf16", "f32r"):
                    vb = vpool.tile([128, D], mm_dt, tag="vb")
                    nc.vector.tensor_copy(vb[:], vt[:])
                    vts.append(vb)
                else:
                    vts.append(vt)

            wts = []
            for s in range(NSLAB):
                z2 = zpool.tile([128, WWIN], f32)
                nc.scalar.activation(
                    z2[:], iota_t[:], AF.Square,
                    bias=sb_t[:, 2 * s + 1 : 2 * s + 2],
                    scale=sb_t[:, 2 * s : 2 * s + 1],
                )
                wt = wpool.tile([128, WWIN], mm_dt)
                nc.scalar.activation(wt[:], z2[:], AF.Exp, scale=-0.5)
                wts.append(wt)

            for t in range(JT):
                ps = pspool.tile([128, D], f32)
                for k in range(3):
                    ls = t + k
                    nc.tensor.matmul(
                        ps[:],
                        wts[ls][:, (2 - k) * 128 : (3 - k) * 128],
                        vts[ls][:],
                        start=(k == 0),
                        stop=(k == 2),
                    )
                ob = opool.tile([128, D], f32)
                nc.vector.tensor_copy(ob[:], ps[:])
                nc.sync.dma_start(out_d[t * 128 : (t + 1) * 128, :], ob[:])

    nc.compile()
    return nc


def _get_nc():
    if DT_MM not in _cached:
        _cached[DT_MM] = build_nc(DT_MM)
    return _cached[DT_MM]


def make_in_maps(V, sigma, mu):
    """Host-side sharding: per-core padded V rows + scale/bias table."""
    V = np.asarray(V, dtype=np.float32)
    sigma = np.asarray(sigma, dtype=np.float32).reshape(B, N)
    mu = np.asarray(mu, dtype=np.float32).reshape(B, N)
    iota_arr = np.ascontiguousarray(
        np.broadcast_to(np.arange(WWIN, dtype=np.float32), (128, WWIN))
    )
    pidx = (np.arange(VROWS) % 128).astype(np.float32)
    in_maps = []
    for c in range(NCORES):
        b, h = divmod(c, 2)
        jb = h * HALF
        lo, hi = jb - 128, jb + HALF + 128
        slo, shi = max(lo, 0), min(hi, N)
        vp = np.zeros((VROWS, D), np.float32)
        sig = np.ones(VROWS, np.float32)
        muv = np.zeros(VROWS, np.float32)
        vp[slo - lo : shi - lo] = V[b, slo:shi]
        sig[slo - lo : shi - lo] = sigma[b, slo:shi]
        muv[slo - lo : shi - lo] = mu[b, slo:shi]
        r = (np.float32(1.0) / sig).astype(np.float32)
        b0 = ((np.float32(-128.0) - pidx - muv) * r).astype(np.float32)
        sb = np.empty((128, 2 * NSLAB), np.float32)
        sb[:, 0::2] = r.reshape(NSLAB, 128).T
        sb[:, 1::2] = b0.reshape(NSLAB, 128).T
        in_maps.append({"Vp": vp, "sb": sb, "iota": iota_arr})
    return in_maps


def gather(results):
    out = np.empty((B, N, D), np.float32)
    for c in range(NCORES):
        b, h = divmod(c, 2)
        out[b, h * HALF : (h + 1) * HALF] = np.asarray(results[c]["out"])
    return out


def kernel(V, sigma, mu):
    nc = _get_nc()
    in_maps = make_in_maps(V, sigma, mu)
    res = run_bass_kernel_spmd(nc, in_maps, core_ids=list(range(NCORES)))
    return gather(res.results)
